# revision 35
# baseline (speedup 1.0000x reference)
"""DifferentialDropout Trainium2 kernel (8-core SPMD), v4.

Reference semantics: per-row corrcoef factor, global-standardized 1000-bin
per-row histograms -> entropies -> per-row keep prob -> mask+scale.

Structure:
  A) stats-only pass over xr (rowsum Pool / sumsq Act / min+max DVE, fused
     accumulators).  xr owns the DMA device, so the stats AllGather fires at
     ~33us and the histogram starts at ~55us.
  B) 32 histogram chunks (prep via the floor-as-round trick; 64 one-hot
     planes split DVE/Act/GpSimd; 512 bf16 32x32-radix matmuls -> PSUM).
     The bf16 cov GEMM runs interleaved: 4 subsweeps (2 m-blocks x 2 n x
     16 k each) using 4 PSUM banks next to the histogram's 4, with xst
     streamed from DRAM in 8KB tiles on the gpsimd queue.  The cov
     ReduceScatter fires ~ch28 and f1 = mean|corr| completes inside B.
  C) batch-count AllGather, entropies, keep/p, masked scale on prefetched
     x/u tiles (compute split DVE/GpSimd).
"""

import sys

sys.path.insert(0, "/opt/trn_rl_repo")

import numpy as np
import os

B = 1024
D = 16384
BINS = 1000
N_CORES = 8
DSL = D // N_CORES      # 2048
RSL = B // N_CORES      # 128
C_ROWS = 4              # rows per histogram chunk
F = C_ROWS * 128        # free elems/partition/chunk = 512
N_CH = RSL // C_ROWS    # 32
LN2 = 0.6931471805599453

# plane assignment: 64 planes (32 H from hi16, 32 L from lo_b)
N_POOL = int(os.environ.get("K_NPOOL", "11"))
N_ACT = int(os.environ.get("K_NACT", "7"))

_cache = {}


def _build():
    import concourse.mybir as mybir
    import concourse.tile as tile
    from concourse import bacc
    from concourse.masks import make_identity

    F32 = mybir.dt.float32
    BF16 = mybir.dt.bfloat16
    I32 = mybir.dt.int32
    I16 = mybir.dt.int16
    A = mybir.AluOpType
    AF = mybir.ActivationFunctionType
    AX = mybir.AxisListType.X
    AXY = mybir.AxisListType.XY

    nc = bacc.Bacc("TRN2", target_bir_lowering=False, debug=False,
                   num_devices=N_CORES)

    xst = nc.dram_tensor("xst", [DSL, B], BF16, kind="ExternalInput")
    xr = nc.dram_tensor("xr", [RSL, D], F32, kind="ExternalInput")
    ur = nc.dram_tensor("ur", [RSL, D], F32, kind="ExternalInput")
    out = nc.dram_tensor("out", [RSL, D], F32, kind="ExternalOutput")

    xr_v = xr.ap().rearrange("r (p e) -> p r e", p=128)   # [128, 128, 128]

    with tile.TileContext(nc) as tc:
        with (
            tc.tile_pool(name="const", bufs=1) as constp,
            tc.tile_pool(name="persist", bufs=1) as persist,
            tc.tile_pool(name="dram", bufs=1, space="DRAM") as dram,
        ):
            id128 = constp.tile([128, 128], F32, name="id128")
            make_identity(nc, id128[:])
            ones32 = constp.tile([32, 1], F32, name="ones32")
            nc.vector.memset(ones32[:], 1.0)
            epsb = constp.tile([128, 1], F32, name="epsb")
            nc.vector.memset(epsb[:], 1e-30)
            one1 = constp.tile([128, 1], F32, name="one1")
            nc.vector.memset(one1[:], 1.0)
            negq = constp.tile([128, 32], F32, name="negq")
            for _q in range(32):
                nc.vector.memset(negq[:, _q:_q + 1], -float(_q))

            # persistent SBUF
            counts_sb = persist.tile([32, RSL * 32], F32, name="counts_sb")
            scal = persist.tile([128, 24], F32, name="scal")
            ag_sb = persist.tile([128, 32], F32, name="ag_sb")
            agg_sb = persist.tile([128, 8, 32], F32, name="agg_sb")
            rowsum = persist.tile([128, 8], F32, name="rowsum")  # full vec
            rsb = persist.tile([128, 1024], F32, name="rsb")
            rdb = persist.tile([128, 1024], F32, name="rdb")
            rs_own = persist.tile([128, 1], F32, name="rs_own")
            rd_own = persist.tile([128, 1], F32, name="rd_own")
            pvec = persist.tile([128, 4], F32, name="pvec")  # p, rkeep, keep, f1

            # DRAM bounces
            ag_in = dram.tile([128, 32], F32, name="ag_in")
            ag_out = dram.tile([1024, 32], F32, addr_space="Shared", name="ag_out")
            cov_in = dram.tile([1024, 1024], F32, name="cov_in")
            cov_out = dram.tile([128, 1024], F32, name="cov_out")
            bcd = dram.tile([32, 32], F32, name="bcd")
            bc_out = dram.tile([256, 32], F32, addr_space="Shared", name="bc_out")

            # ---------------- Phase A: stats only ----------------------------
            with (
                tc.tile_pool(name="pa_io", bufs=3) as pa_io,
                tc.tile_pool(name="pa_w", bufs=2) as pa_w,
            ):
                # rowsum[0:8] (Pool), sumsq[8:16] (Act), min[16:24] max[24:32]
                # (DVE) partials per 2K chunk
                SCH = 2048
                mn_acc = pa_w.tile([128, SCH], F32, name="mn_acc", bufs=1)
                mx_acc = pa_w.tile([128, SCH], F32, name="mx_acc", bufs=1)
                for sc_ in range(8):
                    xrs = pa_io.tile([128, SCH], F32, name="xrs")
                    nc.sync.dma_start(xrs[:], xr.ap()[:, sc_ * SCH:(sc_ + 1) * SCH])
                    rs_scr = pa_w.tile([128, SCH], F32, name="rs_scr", tag="scr",
                                       bufs=2)
                    nc.vector.tensor_scalar(rs_scr[:], xrs[:], 1.0, 0.0, A.mult,
                                            A.add, accum_out=ag_sb[:, sc_:sc_ + 1])
                    sq_scr = pa_w.tile([128, SCH], F32, name="sq_scr", tag="scr",
                                       bufs=2)
                    nc.scalar.activation(sq_scr[:], xrs[:], AF.Square,
                                         accum_out=ag_sb[:, 8 + sc_:9 + sc_])
                    if sc_ == 0:
                        nc.vector.tensor_copy(mn_acc[:], xrs[:])
                        nc.vector.tensor_copy(mx_acc[:], xrs[:])
                    else:
                        nc.vector.tensor_tensor(mn_acc[:], mn_acc[:], xrs[:],
                                                A.min)
                        nc.vector.tensor_max(mx_acc[:], mx_acc[:], xrs[:])
                nc.vector.memset(ag_sb[:, 16:24], 3.4e38)
                nc.vector.memset(ag_sb[:, 24:32], -3.4e38)
                nc.vector.reduce_sum(ag_sb[:, 16:17], mn_acc[:], axis=AX, op=A.min)
                nc.vector.reduce_sum(ag_sb[:, 24:25], mx_acc[:], axis=AX, op=A.max)

                # stats AllGather
                nc.sync.dma_start(ag_in[:], ag_sb[:])
                nc.gpsimd.collective_compute(
                    "AllGather", A.bypass,
                    replica_groups=[list(range(N_CORES))],
                    ins=[ag_in.opt()], outs=[ag_out.opt()])
                nc.sync.dma_start(
                    agg_sb[:], ag_out[:].rearrange("(r p) c -> p r c", p=128))

                # -------- local (own-row) stats --------
                nc.vector.reduce_sum(rs_own[:], ag_sb[:, 0:8], axis=AX)
                ss_own = pa_w.tile([128, 1], F32, name="ss_own", bufs=1)
                nc.vector.reduce_sum(ss_own[:], ag_sb[:, 8:16], axis=AX)
                d_own = pa_w.tile([128, 1], F32, name="d_own", bufs=1)
                nc.vector.tensor_mul(d_own[:], rs_own[:], rs_own[:])
                nc.vector.tensor_scalar(d_own[:], d_own[:], -1.0 / float(D),
                                        ss_own[:], A.mult, A.add)
                nc.scalar.activation(d_own[:], d_own[:], AF.Sqrt)
                nc.vector.reciprocal(rd_own[:], d_own[:])

                # -------- global stats from the AllGather --------
                nc.vector.reduce_sum(rowsum[:], agg_sb[:, :, 0:8], axis=AX)
                nc.vector.reduce_sum(scal[:, 16:17], rowsum[:], axis=AX)
                nc.gpsimd.partition_all_reduce(scal[:, 0:1], scal[:, 16:17], 128,
                                               _reduce_add())
                nc.vector.reduce_sum(scal[:, 17:18], agg_sb[:, :, 8:16], axis=AXY)
                nc.gpsimd.partition_all_reduce(scal[:, 1:2], scal[:, 17:18], 128,
                                               _reduce_add())
                nc.vector.reduce_sum(scal[:, 18:19], agg_sb[:, :, 16:24], axis=AXY,
                                     op=A.min)
                nc.vector.tensor_single_scalar(scal[:, 18:19], scal[:, 18:19],
                                               -1.0, A.mult)
                nc.gpsimd.partition_all_reduce(scal[:, 2:3], scal[:, 18:19], 128,
                                               _reduce_max())
                nc.vector.reduce_sum(scal[:, 19:20], agg_sb[:, :, 24:32], axis=AXY,
                                     op=A.max)
                nc.gpsimd.partition_all_reduce(scal[:, 3:4], scal[:, 19:20], 128,
                                               _reduce_max())

                # -------- scalar constants (SC/BC gate the histogram) --------
                N_f = float(B) * float(D)
                nc.vector.tensor_single_scalar(scal[:, 4:5], scal[:, 0:1],
                                               1.0 / N_f, A.mult)
                nc.vector.tensor_mul(scal[:, 20:21], scal[:, 0:1], scal[:, 4:5])
                nc.vector.tensor_sub(scal[:, 20:21], scal[:, 1:2], scal[:, 20:21])
                nc.vector.tensor_single_scalar(scal[:, 20:21], scal[:, 20:21],
                                               1.0 / (N_f - 1.0), A.mult)
                nc.scalar.activation(scal[:, 5:6], scal[:, 20:21], AF.Sqrt)
                nc.vector.reciprocal(scal[:, 6:7], scal[:, 5:6])
                # lo = (tmin - mu)*rsd ; tmin = -negmn
                nc.vector.tensor_single_scalar(scal[:, 21:22], scal[:, 2:3], -1.0,
                                               A.mult)
                nc.vector.tensor_sub(scal[:, 21:22], scal[:, 21:22], scal[:, 4:5])
                nc.vector.tensor_mul(scal[:, 7:8], scal[:, 21:22], scal[:, 6:7])
                # hi = (tmax - mu)*rsd ; width = (hi - lo)/BINS
                nc.vector.tensor_sub(scal[:, 22:23], scal[:, 3:4], scal[:, 4:5])
                nc.vector.tensor_mul(scal[:, 22:23], scal[:, 22:23], scal[:, 6:7])
                nc.vector.tensor_sub(scal[:, 22:23], scal[:, 22:23], scal[:, 7:8])
                nc.vector.tensor_single_scalar(scal[:, 8:9], scal[:, 22:23],
                                               1.0 / BINS, A.mult)
                nc.vector.reciprocal(scal[:, 9:10], scal[:, 8:9])
                # SC = rsd*rwidth ; BC = -(mu*rsd + lo)*rwidth - 0.5 (floor)
                nc.vector.tensor_mul(scal[:, 10:11], scal[:, 6:7], scal[:, 9:10])
                nc.vector.tensor_mul(scal[:, 23:24], scal[:, 4:5], scal[:, 6:7])
                nc.vector.tensor_add(scal[:, 23:24], scal[:, 23:24], scal[:, 7:8])
                nc.vector.tensor_mul(scal[:, 23:24], scal[:, 23:24], scal[:, 9:10])
                nc.vector.tensor_scalar(scal[:, 11:12], scal[:, 23:24], -1.0, -0.5,
                                        A.mult, A.add)
                # entropy consts: rnw_l = 1/(width*D), rnw_b = 1/(width*N)
                nc.vector.tensor_single_scalar(scal[:, 16:17], scal[:, 8:9],
                                               float(D), A.mult)
                nc.vector.reciprocal(scal[:, 12:13], scal[:, 16:17])
                nc.vector.tensor_single_scalar(scal[:, 17:18], scal[:, 8:9], N_f,
                                               A.mult)
                nc.vector.reciprocal(scal[:, 13:14], scal[:, 17:18])
                nc.vector.tensor_single_scalar(scal[:, 14:15], scal[:, 12:13],
                                               -1.0 / LN2, A.mult)
                nc.vector.tensor_single_scalar(scal[:, 15:16], scal[:, 13:14],
                                               -1.0 / LN2, A.mult)

                # full d-vector / reciprocal, broadcast tables
                ssf = pa_w.tile([128, 8], F32, name="ssf", bufs=1)
                nc.vector.reduce_sum(ssf[:], agg_sb[:, :, 8:16], axis=AX)
                dful = pa_w.tile([128, 8], F32, name="dful", bufs=1)
                nc.vector.tensor_mul(dful[:], rowsum[:], rowsum[:])
                nc.vector.scalar_tensor_tensor(dful[:], dful[:], -1.0 / float(D),
                                               ssf[:], A.mult, A.add)
                nc.scalar.activation(dful[:], dful[:], AF.Sqrt)
                rdc = pa_w.tile([128, 8], F32, name="rdc", bufs=1)
                nc.vector.reciprocal(rdc[:], dful[:])
                with tc.tile_pool(name="bc_ps", bufs=1, space="PSUM") as bc_ps:
                    _bcast_cols(nc, pa_w, bc_ps, rowsum, rsb, id128)
                    _bcast_cols(nc, pa_w, bc_ps, rdc, rdb, id128)

            # ---------------- Phase B: histogram + GEMM + f1 -----------------
            # GEMM subsweeps: s=0..3 covers m-blocks {2s, 2s+1} x n {0,1},
            # 16 k-tiles each, into 4 PSUM banks; MMs interleaved into chunks
            # 7s+1 .. 7s+4 (16 per chunk); xst streamed as [128,4,1024] bf16
            # tiles on the gpsimd queue, one chunk ahead; drained at 7s+6.
            with (
                tc.tile_pool(name="hb_io", bufs=3) as hb_io,
                tc.tile_pool(name="hb_w", bufs=2) as hb_w,
                tc.tile_pool(name="hb_pl", bufs=2) as hb_pl,
                tc.tile_pool(name="hb_xst", bufs=1) as hb_xst,
                tc.tile_pool(name="hb_cov", bufs=2) as hb_cov,
                tc.tile_pool(name="hb_ps", bufs=4, space="PSUM") as hb_ps,
                tc.tile_pool(name="hb_gps", bufs=4, space="PSUM") as hb_gps,
                tc.tile_pool(name="fb_w", bufs=1) as fb_w,
            ):
                sweeps = {}   # s -> dict with psum tiles and stream tiles
                covc = fb_w.tile([128, 1024], F32, name="covc")

                def gemm_stream(s, g):
                    """issue the xst dma for k-group g of subsweep s"""
                    xt = hb_xst.tile([128, 4, 1024], BF16, name="xt")
                    nc.gpsimd.dma_start(
                        xt[:], xst.ap()[g * 512:(g + 1) * 512, :]
                        .rearrange("(k p) c -> p k c", p=128))
                    sweeps.setdefault(s, {})[("xt", g)] = xt

                def gemm_mms_range(s, k0, k1):
                    """emit MMs for k-tiles [k0, k1) of subsweep s"""
                    sw = sweeps[s]
                    if "ps" not in sw:
                        sw["ps"] = {}
                        for mi in range(2):
                            for nn_ in range(2):
                                sw["ps"][(mi, nn_)] = hb_gps.tile(
                                    [128, 512], F32, name="gp")
                    for kk in range(k0, k1):
                        xt = sw[("xt", kk // 4)]
                        k = kk % 4
                        for mi in range(2):
                            mb = 2 * s + mi
                            for nn_ in range(2):
                                nc.tensor.matmul(
                                    sw["ps"][(mi, nn_)][:],
                                    xt[:, k, mb * 128:(mb + 1) * 128],
                                    xt[:, k, nn_ * 512:(nn_ + 1) * 512],
                                    start=(kk == 0), stop=(kk == 15))

                def gemm_drain(s):
                    sw = sweeps[s]
                    for mi in range(2):
                        mb = 2 * s + mi
                        covm = hb_cov.tile([128, 1024], F32, name="covm")
                        for nn_ in range(2):
                            if nn_ == 0:
                                nc.scalar.copy(
                                    covm[:, nn_ * 512:(nn_ + 1) * 512],
                                    sw["ps"][(mi, nn_)][:])
                            else:
                                nc.vector.tensor_copy(
                                    covm[:, nn_ * 512:(nn_ + 1) * 512],
                                    sw["ps"][(mi, nn_)][:])
                        nc.gpsimd.dma_start(
                            cov_in[:].rearrange("(m p) j -> p m j", p=128)
                            [:, mb, :], covm[:])

                gemm_stream(0, 0)
                for ch in range(N_CH):
                    r0 = ch * C_ROWS
                    xch = hb_io.tile([128, C_ROWS, 128], F32, name="xch")
                    nc.sync.dma_start(xch[:], xr_v[:, r0:r0 + C_ROWS, :])
                    xf = xch[:].rearrange("p a b -> p (a b)")

                    vm = hb_w.tile([128, F], F32, name="vm", tag="ew", bufs=3)
                    nc.scalar.activation(vm[:], xf, AF.Identity,
                                         bias=scal[:, 11:12], scale=scal[:, 10:11])
                    i3i = hb_w.tile([128, F], I32, name="i3i", tag="ew", bufs=3)
                    nc.vector.tensor_scalar(i3i[:], vm[:], 998.75, -0.25,
                                            A.min, A.max)
                    hi16 = hb_w.tile([128, F], I16, name="hi16", tag="ew", bufs=3)
                    nc.vector.tensor_scalar(hi16[:], i3i[:], 1.0 / 32.0,
                                            -0.484375, A.mult, A.add)
                    il32 = hb_w.tile([128, F], I32, name="il32", tag="ew", bufs=3)
                    nc.vector.tensor_single_scalar(il32[:], i3i[:], 31,
                                                   A.bitwise_and)
                    lo_b = hb_w.tile([128, F], BF16, name="lo_b", tag="ew", bufs=3)
                    nc.scalar.copy(lo_b[:], il32[:])

                    Hpl = hb_pl.tile([128, 32, F], BF16, name="Hpl")
                    Lpl = hb_pl.tile([128, 32, F], BF16, name="Lpl")

                    def plane_dst(i):
                        return Hpl[:, i, :] if i < 32 else Lpl[:, i - 32, :]

                    def plane_src(i):
                        return hi16[:] if i < 32 else lo_b[:]

                    def plane_val(i):
                        return float(i if i < 32 else i - 32)

                    for i in range(N_POOL):
                        nc.gpsimd.tensor_single_scalar(
                            plane_dst(i), plane_src(i), plane_val(i), A.is_equal)
                    for i in range(N_POOL, N_POOL + N_ACT):
                        vq = int(plane_val(i))
                        atmp = hb_w.tile([128, F], BF16, name="atmp", tag="at",
                                         bufs=2)
                        nc.scalar.activation(atmp[:], plane_src(i), AF.Square,
                                             bias=negq[:, vq:vq + 1])
                        nc.scalar.activation(plane_dst(i), atmp[:], AF.Relu,
                                             bias=one1[:], scale=-1.0)
                    for i in range(N_POOL + N_ACT, 64):
                        nc.vector.tensor_single_scalar(
                            plane_dst(i), plane_src(i), plane_val(i), A.is_equal)

                    for r in range(C_ROWS):
                        ps = hb_ps.tile([32, 32], F32, name="ps")
                        for e in range(128):
                            t = r * 128 + e
                            nc.tensor.matmul(ps[:], Hpl[:, :, t], Lpl[:, :, t],
                                             start=(e == 0), stop=(e == 127))
                        rr = r0 + r
                        if r % 2 == 0:
                            nc.scalar.copy(counts_sb[:, rr * 32:(rr + 1) * 32],
                                           ps[:32, :])
                        else:
                            nc.vector.tensor_copy(
                                counts_sb[:, rr * 32:(rr + 1) * 32], ps[:32, :])

                    # interleaved GEMM work for this chunk position
                    for s in range(4):
                        base = 7 * s
                        if ch == base:
                            gemm_stream(s, 1)
                        if base + 1 <= ch <= base + 4:
                            j = ch - base - 1
                            if j == 0:
                                gemm_stream(s, 2)
                            if j == 1:
                                gemm_stream(s, 3)
                            if j == 2 and s < 3:
                                gemm_stream(s + 1, 0)
                            gemm_mms_range(s, 4 * j, 4 * j + 4)
                        if ch == base + 6:
                            gemm_drain(s)

                    if ch == 28:
                        nc.gpsimd.collective_compute(
                            "ReduceScatter", A.add,
                            replica_groups=[list(range(N_CORES))],
                            ins=[cov_in.opt()], outs=[cov_out.opt()])
                        nc.gpsimd.dma_start(covc[:], cov_out[:])

                    if ch == 31:
                        # f1 = mean|corr| (RS landed ~2 chunks ago)
                        nrs = fb_w.tile([128, 1], F32, name="nrs")
                        nc.vector.tensor_single_scalar(nrs[:], rs_own[:],
                                                       -1.0 / float(D), A.mult)
                        nc.vector.scalar_tensor_tensor(covc[:], rsb[:], nrs[:],
                                                       covc[:], A.mult, A.add)
                        nc.vector.tensor_mul(covc[:], covc[:], rdb[:])
                        nc.vector.tensor_single_scalar(covc[:], covc[:],
                                                       rd_own[:], A.mult)
                        nc.scalar.activation(covc[:], covc[:], AF.Abs,
                                             accum_out=pvec[:, 3:4])
                        nc.vector.tensor_single_scalar(pvec[:, 3:4], pvec[:, 3:4],
                                                       1.0 / float(B), A.mult)

                # batch-count partial -> AllGather (bin = 32q + l)
                bc_part = hb_w.tile([32, 32], F32, name="bc_part", bufs=1)
                nc.vector.reduce_sum(
                    bc_part[:],
                    counts_sb[:].rearrange("p (r l) -> p l r", r=RSL), axis=AX)
                nc.gpsimd.dma_start(bcd[:], bc_part[:])
                nc.gpsimd.collective_compute(
                    "AllGather", A.bypass,
                    replica_groups=[list(range(N_CORES))],
                    ins=[bcd.opt()], outs=[bc_out.opt()])

            # ---------------- Phase C: entropies + mask ----------------------
            with (
                tc.tile_pool(name="pc_w", bufs=2) as pc_w,
                tc.tile_pool(name="pc_big", bufs=1) as pc_big,
                tc.tile_pool(name="pc_ps", bufs=2, space="PSUM") as pc_ps,
                tc.tile_pool(name="pc_io", bufs=4) as pc_io,
            ):
                # prefetch mask-pass inputs (stream during late B / C)
                CH = 2048
                NMC = D // CH
                xm_t = []
                um_t = []
                for c in range(NMC):
                    xm = pc_io.tile([128, CH], F32, name="xm", bufs=4)
                    um = pc_io.tile([128, CH], F32, name="um", bufs=4)
                    nc.gpsimd.dma_start(xm[:], xr.ap()[:, c * CH:(c + 1) * CH])
                    nc.gpsimd.dma_start(um[:], ur.ap()[:, c * CH:(c + 1) * CH])
                    xm_t.append(xm)
                    um_t.append(um)

                # local entropies
                lnch = pc_big.tile([32, RSL * 32], F32, name="lnch")
                nc.scalar.activation(lnch[:], counts_sb[:], AF.Ln,
                                     scale=scal[0:32, 12:13], bias=epsb[0:32, :])
                nc.vector.tensor_mul(lnch[:], lnch[:], counts_sb[:])
                erp = pc_w.tile([32, RSL], F32, name="erp", bufs=1)
                nc.vector.reduce_sum(
                    erp[:], lnch[:].rearrange("p (r l) -> p r l", r=RSL), axis=AX)
                psS = pc_ps.tile([1, RSL], F32, name="psS")
                nc.tensor.matmul(psS[:], ones32[:], erp[:], start=True, stop=True)
                srow = pc_w.tile([1, RSL], F32, name="srow", bufs=1)
                nc.scalar.copy(srow[:], psS[:])
                psT = pc_ps.tile([128, 1], F32, name="psT")
                nc.tensor.transpose(psT[:], srow[:], id128[:1, :1])
                hloc = pc_w.tile([128, 1], F32, name="hloc", bufs=1)
                nc.scalar.copy(hloc[:], psT[:])
                nc.vector.tensor_mul(hloc[:], hloc[:], scal[:, 14:15])

                # batch entropy: sum gathered bc partials
                agb = pc_w.tile([32, 8, 32], F32, name="agb", bufs=1)
                nc.sync.dma_start(
                    agb[:], bc_out[:].rearrange("(r p) c -> p r c", p=32))
                bcs = pc_w.tile([32, 32], F32, name="bcs", bufs=1)
                nc.vector.reduce_sum(
                    bcs[:], agb[:].rearrange("p r c -> p c r"), axis=AX)
                lnb = pc_w.tile([32, 32], F32, name="lnb", bufs=1)
                nc.scalar.activation(lnb[:], bcs[:], AF.Ln,
                                     scale=scal[0:32, 13:14], bias=epsb[0:32, :])
                nc.vector.tensor_mul(lnb[:], lnb[:], bcs[:])
                sb1 = pc_w.tile([32, 1], F32, name="sb1", bufs=1)
                nc.vector.reduce_sum(sb1[:], lnb[:], axis=AX)
                nc.gpsimd.partition_all_reduce(sb1[:], sb1[:], 32, _reduce_add())
                hbat = pc_w.tile([128, 1], F32, name="hbat", bufs=1)
                nc.gpsimd.partition_broadcast(hbat[:], sb1[0:1, :])
                nc.vector.tensor_mul(hbat[:], hbat[:], scal[:, 15:16])

                # f2' = max(f2, 1/f2); keep = f1/f2'; p = 1-keep; rkeep = 1/keep
                tA = pc_w.tile([128, 1], F32, name="tA", bufs=1)
                tB = pc_w.tile([128, 1], F32, name="tB", bufs=1)
                nc.vector.reciprocal(tA[:], hbat[:])
                f2 = pc_w.tile([128, 1], F32, name="f2", bufs=1)
                nc.vector.tensor_mul(f2[:], hloc[:], tA[:])
                nc.vector.reciprocal(tB[:], f2[:])
                nc.vector.tensor_max(f2[:], f2[:], tB[:])
                nc.vector.reciprocal(tB[:], f2[:])
                nc.vector.tensor_mul(pvec[:, 2:3], pvec[:, 3:4], tB[:])
                nc.vector.tensor_scalar(pvec[:, 0:1], pvec[:, 2:3], -1.0, 1.0,
                                        A.mult, A.add)
                nc.vector.reciprocal(pvec[:, 1:2], pvec[:, 2:3])

                # mask + scale on the prefetched tiles (mult split DVE/GpSimd)
                for c in range(NMC):
                    um, xm = um_t[c], xm_t[c]
                    nc.vector.tensor_scalar(um[:], um[:], pvec[:, 0:1],
                                            pvec[:, 1:2], A.is_gt, A.mult)
                    oc = pc_io.tile([128, CH], F32, name="oc", bufs=3)
                    if c % 2 == 0:
                        nc.vector.tensor_mul(oc[:], um[:], xm[:])
                    else:
                        nc.gpsimd.tensor_mul(oc[:], um[:], xm[:])
                    nc.sync.dma_start(out.ap()[:, c * CH:(c + 1) * CH], oc[:])

    nc.compile()
    return nc


def _reduce_add():
    from concourse import bass_isa
    return bass_isa.ReduceOp.add


def _reduce_max():
    from concourse import bass_isa
    return bass_isa.ReduceOp.max


def _bcast_cols(nc, sbuf_pool, psum_pool, vec8, dst, id128):
    """dst[p, t*128+q] = vec8[q, t]  (flatten [128,8] col-major, bcast to all
    partitions)."""
    import concourse.mybir as mybir
    F32 = mybir.dt.float32
    pt = psum_pool.tile([8, 128], F32, name="bc_pt")
    nc.tensor.transpose(pt[:8, :], vec8[:], id128[:])
    tr = sbuf_pool.tile([8, 128], F32, name="bc_tr", bufs=1)
    nc.scalar.copy(tr[:], pt[:8, :])
    flat = sbuf_pool.tile([1, 8 * 128], F32, name="bc_flat", bufs=1)
    for t in range(8):
        nc.sync.dma_start(flat[:, t * 128:(t + 1) * 128], tr[t:t + 1, :])
    nc.gpsimd.partition_broadcast(dst[:], flat[:])


def _to_bf16(a):
    """numpy f32 -> bf16 (round-to-nearest-even) via ml_dtypes."""
    import ml_dtypes
    return a.astype(ml_dtypes.bfloat16)


def kernel(x, u):
    if "nc" not in _cache:
        _cache["nc"] = _build()
    nc = _cache["nc"]
    from concourse.bass_utils import run_bass_kernel_spmd

    x = np.asarray(x, dtype=np.float32)
    u = np.asarray(u, dtype=np.float32)
    orig_shape = x.shape
    xf = np.ascontiguousarray(x.reshape(B, D))
    uf = np.ascontiguousarray(u.reshape(B, D))
    in_maps = []
    for c in range(N_CORES):
        in_maps.append({
            "xst": _to_bf16(np.ascontiguousarray(xf[:, c * DSL:(c + 1) * DSL].T)),
            "xr": np.ascontiguousarray(xf[c * RSL:(c + 1) * RSL, :]),
            "ur": np.ascontiguousarray(uf[c * RSL:(c + 1) * RSL, :]),
        })
    res = run_bass_kernel_spmd(nc, in_maps, core_ids=list(range(N_CORES)))
    _cache["last_results"] = res
    outf = np.concatenate([res.results[c]["out"] for c in range(N_CORES)], axis=0)
    return outf.reshape(orig_shape)
